# revision 5
# baseline (speedup 1.0000x reference)
"""Multi-head attention Trainium2 Bass kernel (v2).

Problem: B=8, S=1024, E=768, H=12, DH=64 MHA with per-head Q/K/V projections
and output projection. Data-parallel over batch: one batch element per
NeuronCore (8 cores).

Per-core dataflow (contraction dim on partitions; bf16 PE operands, fp32
PSUM):
  xT [E,S] bf16  <- DMA-transpose of x (6 tiles)
  v_sb[t] [128(t), 6 pairs x (64|1|64|1)] <- direct matmul xT.T @ Wv per
     s-tile (no PE transposes); ones columns for the softmax denominator;
     bias via Pool TT with broadcast bv.
  qT/kT = W.T @ xT + b per head-pair [128(d-pair), S] bf16 (per-partition
     bias via DVE tensor_scalar_add on the PSUM->SBUF copy)
  scoresT[t,s] = k @ q.T per head, K=64 matmuls (two heads share the PE
     array via tile_position row strips); one [128,512] PSUM bank per
     (t,e,ch) chunk, scp bufs=4 for fine-grained recycle
  expST = exp(0.125*scoresT) (ACT, bf16; no max subtraction: scores ~ N(0,1))
  attnT[d,s| Z] = [v|1].T @ expST
  catT = attnT * (1/Z): DVE reciprocal + gpsimd partition_broadcast (SBUF
     only, no DRAM round trip) + Pool TT mult
  out = catT.T @ Wo, bias folded into the PSUM->SBUF copy via TT with
     broadcast bo (no bias matmuls); one [128,768] f32 DMA per s-tile
"""
import sys

sys.path.insert(0, "/opt/trn_rl_repo")

import numpy as np
import ml_dtypes
from contextlib import ExitStack

import concourse.bass as bass
import concourse.tile as tile
from concourse import bacc, mybir
from concourse.bass_utils import run_bass_kernel_spmd

F32 = mybir.dt.float32
BF16 = mybir.dt.bfloat16
AF = mybir.ActivationFunctionType
ALU = mybir.AluOpType
BF = ml_dtypes.bfloat16

B, S, E, H, DH = 8, 1024, 768, 12, 64
NP_ = 6          # head pairs
ET = 6           # e tiles of 128
ST = 8           # s tiles of 128
NCORES = 8

_cache = {}


def _build_nc(reps=1, ablate=""):
    if ("nc", reps, ablate) in _cache:
        return _cache[("nc", reps, ablate)]
    nc = bacc.Bacc("TRN2", target_bir_lowering=False, debug=False,
                   num_devices=NCORES)

    x = nc.dram_tensor("x", [S, E], BF16, kind="ExternalInput").ap()
    wqk = nc.dram_tensor("wqk", [NP_, 128, 2, ET, 128], BF16,
                         kind="ExternalInput").ap()
    wv = nc.dram_tensor("wv", [128, ET, E], BF16, kind="ExternalInput").ap()
    bqk = nc.dram_tensor("bqk", [128, NP_, 2], F32, kind="ExternalInput").ap()
    bv = nc.dram_tensor("bv", [1, E], F32, kind="ExternalInput").ap()
    wo = nc.dram_tensor("wo", [128, ET * E], BF16, kind="ExternalInput").ap()
    bo = nc.dram_tensor("bo", [1, E], F32, kind="ExternalInput").ap()
    out = nc.dram_tensor("out", [S, E], F32, kind="ExternalOutput").ap()

    exq_bufs = 2 if "exq2" in ablate else 3
    with tile.TileContext(nc) as tc, ExitStack() as ctx:
        consts = ctx.enter_context(tc.tile_pool(name="consts", bufs=1))
        xtp = ctx.enter_context(tc.tile_pool(name="xtp", bufs=1))
        vsb = ctx.enter_context(tc.tile_pool(name="vsb", bufs=1))
        catp = ctx.enter_context(tc.tile_pool(name="catp", bufs=1))
        wqkp = ctx.enter_context(tc.tile_pool(name="wqkp", bufs=3))
        qkp = ctx.enter_context(tc.tile_pool(name="qkp", bufs=3))
        exq = ctx.enter_context(tc.tile_pool(name="exq", bufs=exq_bufs))
        zp = ctx.enter_context(tc.tile_pool(name="zp", bufs=4))
        cup = ctx.enter_context(tc.tile_pool(name="cup", bufs=2))
        osb = ctx.enter_context(tc.tile_pool(name="osb", bufs=2))
        # PSUM: 2 + 4 + 2 = 8 banks
        mmp = ctx.enter_context(tc.tile_pool(name="mmp", bufs=2, space="PSUM"))
        scp = ctx.enter_context(tc.tile_pool(name="scp", bufs=4, space="PSUM"))
        atp = ctx.enter_context(tc.tile_pool(name="atp", bufs=2, space="PSUM"))

        wv_t = consts.tile([128, ET, E], BF16, tag="wv")
        nc.sync.dma_start(wv_t, wv)
        wo_t = consts.tile([128, ET * E], BF16, tag="wo")
        nc.sync.dma_start(wo_t, wo)
        bqk_t = consts.tile([128, NP_, 2], F32, tag="bqk")
        nc.sync.dma_start(bqk_t, bqk)
        bv_row = consts.tile([1, E], F32, tag="bv_row")
        nc.sync.dma_start(bv_row, bv)
        bo_row = consts.tile([1, E], F32, tag="bo_row")
        nc.sync.dma_start(bo_row, bo)
        bv_rep = consts.tile([128, E], F32, tag="bv_rep")
        nc.gpsimd.partition_broadcast(bv_rep, bv_row)
        bo_rep = consts.tile([128, E], F32, tag="bo_rep")
        nc.gpsimd.partition_broadcast(bo_rep, bo_row)

        for _rep in range(reps):
            # ---- Phase 0: xT [E, S] via DMA transpose ----
            xT = [xtp.tile([128, S], BF16, tag=f"xT{et}", name=f"xT{et}")
                  for et in range(ET)]
            for et in range(ET):
                nc.sync.dma_start(
                    xT[et], x[:, et * 128:(et + 1) * 128], transpose=True)

            # ---- Phase 0.5: v_sb[t] [128, NP_, 2, 65] direct ----
            v_sb = []
            for t in range(ST):
                vt = vsb.tile([128, NP_, 2, 65], BF16, tag=f"v{t}",
                              name=f"v{t}")
                v_sb.append(vt)
                nc.gpsimd.memset(vt[:, :, :, 64:65], 1.0)
                for ch in range(2):
                    pv = mmp.tile([128, 512], F32, tag="mm", name="pv")
                    for et in range(ET):
                        nc.tensor.matmul(
                            pv[:, 0:384],
                            xT[et][:, t * 128:(t + 1) * 128],
                            wv_t[:, et, ch * 384:(ch + 1) * 384],
                            start=(et == 0), stop=(et == ET - 1),
                        )
                    dst = vt[:, 3 * ch:3 * ch + 3, :, 0:64]
                    src = pv[:, 0:384].rearrange(
                        "p (pr e d) -> p pr e d", pr=3, e=2)
                    brep = bv_rep[:, ch * 384:(ch + 1) * 384].rearrange(
                        "p (pr e d) -> p pr e d", pr=3, e=2)
                    nc.vector.tensor_tensor(
                        out=dst, in0=src, in1=brep, op=ALU.add)

            catT = [catp.tile([128, S], BF16, tag=f"catT{j}", name=f"catT{j}")
                    for j in range(NP_)]

            # ---- Per head-pair, software-pipelined (lag 2) ----
            def produce(p):
                wt = wqkp.tile([128, 2, ET, 128], BF16, tag="wqk", name="wt")
                nc.sync.dma_start(wt, wqk[p])

                qT = qkp.tile([128, S], BF16, tag="qT", name="qT")
                kT = qkp.tile([128, S], BF16, tag="kT", name="kT")
                for qi, dst in ((0, qT), (1, kT)):
                    for ch in range(2):
                        pp = mmp.tile([128, 512], F32, tag="mm", name="pp")
                        for et in range(ET):
                            nc.tensor.matmul(
                                pp, wt[:, qi, et, :],
                                xT[et][:, ch * 512:(ch + 1) * 512],
                                start=(et == 0), stop=(et == ET - 1),
                            )
                        nc.vector.tensor_scalar_add(
                            dst[:, ch * 512:(ch + 1) * 512], pp,
                            bqk_t[:, p, qi:qi + 1])

                if ablate == "noattn":
                    for e in range(2):
                        nc.vector.memset(catT[p][64 * e:64 * e + 64, :], 0.5)
                    return None
                # scores for both heads (K=64 row strips), one PSUM bank
                # per (t, e, ch) chunk + immediate exp
                ex_ts = [exq.tile([128, ST, S], BF16, tag=f"ex{e}",
                                  name=f"ex{e}") for e in range(2)]
                for t in range(ST):
                    scs = []
                    for e in range(2):
                        r0 = 64 * e
                        for ch in range(2):
                            sc = scp.tile([128, 512], F32, tag="sc",
                                          name="sc")
                            scs.append((e, ch, sc))
                            nc.tensor.matmul(
                                sc,
                                kT[r0:r0 + 64, t * 128:(t + 1) * 128],
                                qT[r0:r0 + 64, ch * 512:(ch + 1) * 512],
                                tile_position=(r0, 0),
                                start=True, stop=True,
                                skip_group_check=True,
                            )
                    if ablate == "nosm":
                        continue
                    for e, ch, sc in scs:
                        nc.scalar.activation(
                            ex_ts[e][:, t, ch * 512:(ch + 1) * 512], sc,
                            AF.Exp, scale=0.125)
                if ablate in ("nosm", "noatmm"):
                    for e in range(2):
                        nc.vector.memset(catT[p][64 * e:64 * e + 64, :], 0.5)
                    return None
                return ex_ts

            def consume(p, ex_ts):
                if ex_ts is None:
                    return
                # Stage unnormalized attention into catU and release each
                # attention PSUM tile quickly (recip + copy only); the
                # broadcast and the normalizing TT run off the critical path.
                catU = cup.tile([128, S], BF16, tag="catU", name="catU")
                deferred = []
                for e in range(2):
                    r0 = 64 * e
                    ex_t = ex_ts[e]
                    for ch in range(2):
                        ap_ = atp.tile([65, 512], F32, tag="att", name="att")
                        for t in range(ST):
                            nc.tensor.matmul(
                                ap_, v_sb[t][:, p, e, :],
                                ex_t[:, t, ch * 512:(ch + 1) * 512],
                                start=(t == 0), stop=(t == ST - 1),
                            )
                        zrec = zp.tile([1, 512], BF16, tag="zrec",
                                       name="zrec", bufs=4)
                        with tc.high_priority(offset=150), \
                                nc.allow_low_precision(
                                    reason="bf16 1/Z, ~0.2% rms"):
                            nc.vector.reciprocal(zrec, ap_[64:65, :])
                            nc.vector.tensor_copy(
                                catU[r0:r0 + 64, ch * 512:(ch + 1) * 512],
                                ap_[0:64, :])
                        zrep = zp.tile([128, 512], BF16, tag="zrep",
                                       name="zrep", bufs=4)
                        nc.gpsimd.partition_broadcast(zrep, zrec)
                        deferred.append((r0, ch, zrep))
                for r0, ch, zrep in deferred:
                    nc.gpsimd.tensor_tensor(
                        out=catT[p][r0:r0 + 64, ch * 512:(ch + 1) * 512],
                        in0=catU[r0:r0 + 64, ch * 512:(ch + 1) * 512],
                        in1=zrep[r0:r0 + 64, :],
                        op=ALU.mult,
                    )

            if "lag1" not in ablate:
                states = {}
                for p in range(NP_):
                    states[p] = produce(p)
                    if p >= 2:
                        consume(p - 2, states.pop(p - 2))
                consume(NP_ - 2, states.pop(NP_ - 2))
                consume(NP_ - 1, states.pop(NP_ - 1))
            else:
                prev = None
                for p in range(NP_):
                    state = produce(p)
                    if p >= 1:
                        consume(p - 1, prev)
                    prev = state
                consume(NP_ - 1, prev)

            # ---- Output projection (bias folded into the PSUM drain) ----
            for st in range(ST):
                o_sb = osb.tile([128, E], F32, tag="ot", name="ot")
                for ch in range(2):
                    op_ = mmp.tile([128, 512], F32, tag="mm", name="op")
                    for j in range(NP_):
                        nc.tensor.matmul(
                            op_[:, 0:384],
                            catT[j][:, st * 128:(st + 1) * 128],
                            wo_t[:, j * E + ch * 384:j * E + ch * 384 + 384],
                            start=(j == 0), stop=(j == NP_ - 1),
                        )
                    nc.vector.tensor_tensor(
                        out=o_sb[:, ch * 384:(ch + 1) * 384],
                        in0=op_[:, 0:384],
                        in1=bo_rep[:, ch * 384:(ch + 1) * 384],
                        op=ALU.add)
                nc.sync.dma_start(
                    out[st * 128:(st + 1) * 128, :], o_sb)

    nc.compile()
    _cache[("nc", reps, ablate)] = nc
    return nc


def _prep_weights(Wq, bq, Wk, bk, Wv, bv, Wo, bo):
    def pack_w(W):  # [12, 768, 64] -> [6, 128, 6, 128] bf16
        Wp = W.reshape(NP_, 2, E, DH).transpose(0, 2, 1, 3).reshape(NP_, E, 128)
        return Wp.reshape(NP_, ET, 128, 128).transpose(0, 2, 1, 3)

    wqk = np.ascontiguousarray(
        np.stack([pack_w(Wq), pack_w(Wk)], axis=2)).astype(BF)

    # wv: [12, 768, 64] -> [768(e), 768(d=h*64+dd)] -> [128, ET, 768]
    wv_flat = Wv.transpose(1, 0, 2).reshape(E, E)
    wv_p = np.ascontiguousarray(
        wv_flat.reshape(ET, 128, E).transpose(1, 0, 2)).astype(BF)

    bqk = np.ascontiguousarray(
        np.stack([bq.reshape(NP_, 128).T, bk.reshape(NP_, 128).T],
                 axis=2).reshape(128, NP_, 2)).astype(np.float32)

    return {
        "wqk": wqk,
        "wv": wv_p,
        "bqk": bqk,
        "bv": np.ascontiguousarray(bv.reshape(1, E)).astype(np.float32),
        "wo": np.ascontiguousarray(
            Wo.reshape(ET, 128, E).transpose(1, 0, 2).reshape(128, ET * E)
        ).astype(BF),
        "bo": np.ascontiguousarray(bo.reshape(1, E)).astype(np.float32),
    }


def kernel(hidden_state, Wq, bq, Wk, bk, Wv, bv, Wo, bo):
    hidden_state = np.asarray(hidden_state, dtype=np.float32)
    shared = _prep_weights(
        np.asarray(Wq, np.float32), np.asarray(bq, np.float32),
        np.asarray(Wk, np.float32), np.asarray(bk, np.float32),
        np.asarray(Wv, np.float32), np.asarray(bv, np.float32),
        np.asarray(Wo, np.float32), np.asarray(bo, np.float32))
    nc = _build_nc()
    in_maps = [
        {"x": np.ascontiguousarray(hidden_state[b]).astype(BF), **shared}
        for b in range(NCORES)
    ]
    res = run_bass_kernel_spmd(nc, in_maps, core_ids=list(range(NCORES)))
    return np.stack([r["out"] for r in res.results], axis=0)
